# revision 53
# baseline (speedup 1.0000x reference)
"""Multi-head attention (B=2, S=2048, D=1024, H=16) on 8 Trainium2 NeuronCores.

Sharding (per the batch+head hint): core c handles batch b=c//4 and head-group
g=c%4 (4 heads, i.e. a 256-column slice of the QKV projections and a 256-row
slice of Wo).  Each core computes q^T/k^T/v projections for its head group,
flash-style attention in transposed score space (scores^T = k^T-tile.T @ q^T,
softmax denominator via a ones-augmented V column in the PV matmul), and its
out-projection partial  ctx_g @ Wo[256g:256(g+1), :].

The out_proj reduction over the 4 head-group cores of each batch is done on
the host (device collectives on this stack cost ~145us for 8MB - far more
than the arithmetic they replace - so the partial-sum gather IS the unshard
step).  Biases: bq/bk are applied on device (they feed the softmax
nonlinearly); bv/bo commute through attention/out_proj linearly and are folded
into a single host-side correction vector  c = bv @ Wo + bo.

Performance structure: the PE array is the binding engine (measured: every
PE micro-stall drops it to the 1.2GHz mid p-state for ~3us, so emission is
software-pipelined to keep it continuously fed):
 - all matmul operands are bf16 (1 col/cycle streaming at 2.4GHz;
   LDWEIGHTS pipelines behind the previous matmul, back-to-back cadence is
   ~N/2.4GHz + ~0).
 - both heads' scores for one sk-tile share a single [128,1024] psum tile,
   so the 2-buffer pool double-buffers across iterations and one exp
   instruction covers both heads.
 - PV matmuls lag the scores/exp stream by LAG iterations; v-projection,
   out-projection of chunk j-1 and q-projection of chunk j+1 are sprinkled
   into the attention loop to fill exp-wait gaps.
 - softmax normalization is fully deferred off the PE path: the psum accs
   are copied to SBUF immediately (freeing their banks for the next head
   pair), the two denominator rows are reshaped to [128,8] by DMA (DVE
   reciprocal cost scales with per-lane free size), recip'd, and broadcast
   across partitions via a DRAM stride-0 bounce on the GpSimd DGE queue.
 - inputs/weights are host-relaid so every DMA reads contiguous 4-8KB runs
   per partition (strided 1-2KB descriptors only reach ~100GB/s), and load
   posts are ordered/gated so DMA fair-sharing cannot starve the transfer
   the PE needs next.
"""

import numpy as np
import ml_dtypes

import concourse.bass as bass
import concourse.mybir as mybir
import concourse.tile as tile
from concourse import bacc
from concourse.bass_utils import run_bass_kernel_spmd

B, S, D, H = 2, 2048, 1024, 16
HD = D // H          # 64 head dim
NCORE = 8
G = NCORE // B       # 4 head-groups per batch
HG = H // G          # 4 heads per group
DG = D // G          # 256 projection columns per group
P = 128              # partitions
KT = D // P          # 8 contraction tiles for projections
CH = 512             # s-chunk (projection rhs width & attention sq chunk)
NJ = S // CH         # 4 chunks
STILES = S // P      # 16 sk tiles
TB = 2               # scores t-batch per exp op (psum: [128, TB*512] = 2 banks)
VBLK = HD + 1        # v block: 64 v cols + 1 ones col (softmax denominator)
LAG = 3              # PV emission lags scores/exp by this many tb-iterations

f32 = mybir.dt.float32
bf16 = mybir.dt.bfloat16
MM_DT = mybir.dt.bfloat16
NP_MM = np.float32 if MM_DT == mybir.dt.float32r else ml_dtypes.bfloat16
EXP = mybir.ActivationFunctionType.Exp
SCALE = 1.0 / np.sqrt(np.float32(HD))


def _build_program():
    nc = bacc.Bacc("TRN2", target_bir_lowering=False, debug=False,
                   num_devices=NCORE)

    # inputs/weights are pre-laid-out on the host so every DMA reads
    # contiguous 4-8KB runs per partition (strided 1KB-run descriptors only
    # reach ~100GB/s on these queues)
    xqT_d = nc.dram_tensor("xqT", [P, NJ * KT * CH], MM_DT, kind="ExternalInput")
    xkT_d = nc.dram_tensor("xkT", [P, NJ * KT * CH], MM_DT, kind="ExternalInput")
    xvT_d = nc.dram_tensor("xvT", [P, NJ * KT * CH], MM_DT, kind="ExternalInput")
    wq_d = nc.dram_tensor("wq", [P, KT * DG], MM_DT, kind="ExternalInput")
    wk_d = nc.dram_tensor("wk", [P, KT * DG], MM_DT, kind="ExternalInput")
    wv_d = nc.dram_tensor("wv", [P, KT * DG], MM_DT, kind="ExternalInput")
    wo_d = nc.dram_tensor("wo", [P, 2 * D], MM_DT, kind="ExternalInput")
    bq_d = nc.dram_tensor("bqk", [P, 4], f32, kind="ExternalInput")
    out_d = nc.dram_tensor("out", [S, D], f32, kind="ExternalOutput")

    with tile.TileContext(nc) as tc:
        _emit(nc, tc, xqT_d, xkT_d, xvT_d, wq_d, wk_d, wv_d, wo_d, bq_d, out_d)
    nc.compile()
    return nc


def _emit(nc, tc, xqT_d, xkT_d, xvT_d, wq_d, wk_d, wv_d, wo_d, bq_d, out_d):
    from collections import deque
    from contextlib import ExitStack
    ctx = ExitStack()
    with ctx:
        consts = ctx.enter_context(tc.tile_pool(name="consts", bufs=1))
        persist = ctx.enter_context(tc.tile_pool(name="persist", bufs=1))
        xpool = ctx.enter_context(tc.tile_pool(name="xchunk", bufs=6))
        epool = ctx.enter_context(tc.tile_pool(name="exps", bufs=10))
        small = ctx.enter_context(tc.tile_pool(name="small", bufs=8))
        bpool = ctx.enter_context(tc.tile_pool(name="bcast", bufs=4))
        opool = ctx.enter_context(tc.tile_pool(name="ostage", bufs=3))
        drbp = ctx.enter_context(tc.tile_pool(name="drb", bufs=4, space="DRAM"))
        ps_s = ctx.enter_context(tc.tile_pool(name="ps_s", bufs=2, space="PSUM"))
        ps_acc = ctx.enter_context(tc.tile_pool(name="ps_acc", bufs=2, space="PSUM"))
        ps_x = ctx.enter_context(tc.tile_pool(name="ps_x", bufs=2, space="PSUM"))

        def load_xchunk(x_d, j, eng=None):
            # spread input streams across DGE queues so transfers overlap
            t = xpool.tile([P, KT * CH], MM_DT, tag="xchunk", name="xchunk")
            (eng or nc.sync).dma_start(
                out=t[:],
                in_=x_d.rearrange("p (j x) -> p j x", j=NJ)[:, j])
            return t.rearrange("p (kt s) -> p kt s", kt=KT)

        # ---- constants: wk first (k-projection starts the kernel).
        # Remaining loads are posted between k-proj chunks (see head phase):
        # outstanding DMAs fair-share bandwidth, so posting everything up
        # front makes the first-needed transfer finish last.
        wk_sb = consts.tile([P, KT * DG], MM_DT, tag="wk")
        nc.sync.dma_start(out=wk_sb[:], in_=wk_d.ap())
        bqk_sb = consts.tile([P, 4], f32, tag="bqk")  # [bq|bk] x m-half
        nc.sync.dma_start(out=bqk_sb[:], in_=bq_d.ap())
        xc0 = load_xchunk(xkT_d, 0)
        wq_sb = consts.tile([P, KT * DG], MM_DT, tag="wq")
        wv_sb = consts.tile([P, KT * DG], MM_DT, tag="wv")
        wo_sb = consts.tile([P, 2 * D], MM_DT, tag="wo")  # 2 k-tiles [128, D]
        # persistent activations
        qT = [persist.tile([P, S], MM_DT, tag=f"qT{m}", name=f"qT{m}")
              for m in range(2)]
        kT = [persist.tile([P, S], MM_DT, tag=f"kT{m}", name=f"kT{m}")
              for m in range(2)]
        v_sb = persist.tile([P, HG * STILES * VBLK], MM_DT, tag="v")
        ctxT = [persist.tile([P, S], MM_DT, tag=f"ctxT{m}", name=f"ctxT{m}")
                for m in range(2)]
        v_view = v_sb.rearrange("p (h t c) -> p h t c", h=HG, t=STILES)

        # ones columns for the softmax-denominator rows of the PV matmuls
        nc.vector.memset(v_view[:, :, :, HD], 1.0)


        def proj_qk(xc, w_sb, dst, bias_i, j):
            # dst[m][dq, j*CH:+CH] = (W[:, m-half].T @ x^T-chunk) + bias
            for m in range(2):
                acc = ps_x.tile([P, CH], f32, tag="px", name="px")
                for k in range(KT):
                    nc.tensor.matmul(
                        acc[:], w_sb[:, k * DG + m * P:k * DG + (m + 1) * P],
                        xc[:, k, :], start=(k == 0), stop=(k == KT - 1))
                nc.vector.tensor_add(
                    dst[m][:, j * CH:(j + 1) * CH], acc[:],
                    bqk_sb[:, 2 * bias_i + m:2 * bias_i + m + 1].broadcast_to(
                        [P, CH]))

        def emit_vproj_unit(xc, j, si):
            # one v s-subtile of 128 rows; heads land in v_view blocks.
            # These are the first sprinkle units: the j0 attention loop is
            # PE-bound while they drain, and PV(t) (lagging by LAG iters)
            # finds its v tile just in time.
            def fn():
                st = j * (CH // P) + si
                acc = ps_x.tile([P, DG], f32, tag="px", name="px")
                for k in range(KT):
                    nc.tensor.matmul(
                        acc[:], xc[:, k, si * P:(si + 1) * P],
                        wv_sb[:, k * DG:(k + 1) * DG],
                        start=(k == 0), stop=(k == KT - 1))
                nc.vector.tensor_copy(
                    v_view[:, :, st, 0:HD],
                    acc[:].rearrange("p (h c) -> p h c", h=HG))
            return fn

        # ---- head phase: k (full S), q chunk 0, v (full S) -------------
        # xq/xv ride the Activation/GpSimd DGE queues so their transfers
        # overlap the xk stream on the SP queue; posts are interleaved with
        # the k-proj chunks in the order the data is needed.
        proj_qk(xc0, wk_sb, kT, 1, 0)
        xc = load_xchunk(xkT_d, 1)
        # gate the scalar-queue loads on k-proj j0 so their transfers don't
        # fair-share DMA bandwidth away from the critical wk/xk stream
        gate_sc = small.tile([1, 4], f32, tag="gate", name="gate_sc")
        nc.scalar.activation(gate_sc[:], kT[1][0:1, 0:4],
                             mybir.ActivationFunctionType.Copy)
        nc.scalar.dma_start(out=wq_sb[:], in_=wq_d.ap())
        xq0 = load_xchunk(xqT_d, 0, eng=nc.scalar)
        proj_qk(xc, wk_sb, kT, 1, 1)
        xc = load_xchunk(xkT_d, 2)
        gate_gp = small.tile([1, 4], f32, tag="gate", name="gate_gp")
        nc.gpsimd.tensor_copy(gate_gp[:], kT[1][0:1, CH:CH + 4])
        nc.gpsimd.dma_start(out=wv_sb[:], in_=wv_d.ap())
        xvc = [load_xchunk(xvT_d, 0, eng=nc.gpsimd)]
        proj_qk(xc, wk_sb, kT, 1, 2)
        xc = load_xchunk(xkT_d, 3)
        xvc.append(load_xchunk(xvT_d, 1, eng=nc.gpsimd))
        proj_qk(xc, wk_sb, kT, 1, 3)
        nc.sync.dma_start(out=wo_sb[:], in_=wo_d.ap())
        proj_qk(xq0, wq_sb, qT, 0, 0)
        xvc.append(load_xchunk(xvT_d, 2, eng=nc.gpsimd))
        xvc.append(load_xchunk(xvT_d, 3, eng=nc.gpsimd))

        # ---- deferred-work queues --------------------------------------
        # pvq: ordered PV/normalize closures (softmax-side pipeline, popped
        #      1 per tb-iteration, lagging production by LAG iterations).
        # sprinkle: independent PE work (out-proj of j-1, q-proj of j+1)
        #      popped up to 2 units per iteration to fill exp-wait gaps.
        pvq = deque()
        sprinkle = deque()
        tail_fill = deque()

        def pop_pv():
            if len(pvq) > LAG:
                fn = pvq.popleft()
                again = fn()
                if again and len(pvq) > LAG:  # normalize is cheap: pop 2
                    pvq.popleft()()

        def pop_sprinkle(n=2):
            for _ in range(min(n, len(sprinkle))):
                sprinkle.popleft()()

        def emit_outproj_unit(jj, si, nh, tail=False, fill=False):
            def fn():
                st = jj * (CH // P) + si
                if tail and (si + nh) % 2 == 0:
                    # scores psum is free during the drain: alternating pools
                    # gives the tail units a 4-deep pipeline
                    po = ps_s.tile([P, 2 * CH], f32, tag="s", name="s")[:, 0:CH]
                else:
                    po = ps_x.tile([P, CH], f32, tag="px", name="px")
                for m in range(2):
                    nc.tensor.matmul(
                        po[:], ctxT[m][:, st * P:(st + 1) * P],
                        wo_sb[:, m * D + nh * CH:m * D + (nh + 1) * CH],
                        start=(m == 0), stop=(m == 1))
                ostage = opool.tile([P, CH], f32, tag="ostage", name="ostage")
                if tail:
                    nc.scalar.activation(ostage[:], po[:],
                                         mybir.ActivationFunctionType.Copy)
                else:
                    nc.vector.tensor_copy(ostage[:], po[:])
                if fill:
                    oeng = nc.gpsimd
                elif tail:
                    oeng = (nc.scalar, nc.sync, nc.gpsimd)[(2 * si + nh) % 3]
                else:
                    oeng = nc.sync
                oeng.dma_start(
                    out_d[st * P:(st + 1) * P, nh * CH:(nh + 1) * CH],
                    ostage[:])
            return fn

        def emit_qproj_unit(xc, m, jj):
            def fn():
                acc = ps_x.tile([P, CH], f32, tag="px", name="px")
                for k in range(KT):
                    nc.tensor.matmul(
                        acc[:], wq_sb[:, k * DG + m * P:k * DG + (m + 1) * P],
                        xc[:, k, :], start=(k == 0), stop=(k == KT - 1))
                nc.vector.tensor_add(
                    qT[m][:, jj * CH:(jj + 1) * CH], acc[:],
                    bqk_sb[:, m:m + 1].broadcast_to([P, CH]))
            return fn

        for jv in range(NJ):
            for si in range(CH // P):
                sprinkle.append(emit_vproj_unit(xvc[jv], jv, si))

        # ---- attention: global software-pipelined iteration stream -----
        for j in range(NJ):
            jc = slice(j * CH, (j + 1) * CH)
            if j + 1 < NJ:
                xq_next = load_xchunk(xqT_d, j + 1)
                xq_stage = xq_next

            for m in range(2):
                # q-proj of j+1 staged at m=1 so its input DMA has landed
                if m == 1 and j + 1 < NJ:
                    for mm_ in range(2):
                        sprinkle.append(emit_qproj_unit(xq_stage, mm_, j + 1))
                # psum accumulators for this head pair (ctx 0-63, den 64)
                accs = [ps_acc.tile([P, CH], f32, tag="acc", name=f"acc{i}")
                        for i in range(2)]
                avs = [accs[0][0:VBLK, :], accs[1][0:VBLK, :]]

                for t in range(STILES):
                    # both heads' scores for sk-tile t in ONE psum tile so
                    # the pool's 2 bufs give true double-buffering, and one
                    # exp instruction covers both heads
                    sps = ps_s.tile([P, 2 * CH], f32, tag="s", name="s")
                    for hh in range(2):
                        lo, hi = hh * HD, (hh + 1) * HD
                        nc.tensor.matmul(
                            sps[:, hh * CH:(hh + 1) * CH],
                            kT[m][lo:hi, t * P:(t + 1) * P],
                            qT[m][lo:hi, jc], start=True, stop=True)
                    e = epool.tile([P, 2 * CH], MM_DT, tag="e", name="e")
                    nc.scalar.activation(e[:], sps[:], EXP, scale=SCALE)

                    def emit_pv(m=m, t=t, e=e, avs=avs):
                        for hh in range(2):
                            nc.tensor.matmul(
                                avs[hh], v_view[:, 2 * m + hh, t, :],
                                e[:, hh * CH:(hh + 1) * CH],
                                start=(t == 0), stop=(t == STILES - 1))
                        return False
                    pvq.append(emit_pv)
                    pop_pv()
                    # keep the DVE queue clear in the boundary window: the
                    # next norm's acc-freeing copies must not sit behind a
                    # burst of out-proj staging copies (j0 still needs its
                    # v-proj units popped immediately)
                    if j == 0 or t >= 4:
                        pop_sprinkle()

                def emit_norm(m=m, j=j, jc=jc, accs=accs):
                    # normalize ctx by the softmax denominators (psum row 64
                    # of each acc).  The accs are copied to SBUF first so
                    # their psum banks free quickly (the next head-pair's PV
                    # reuses them).  DVE reciprocal cost scales with per-lane
                    # free size, so the two [1,512] denominator rows are
                    # reshaped to [128,8] (DMA crosses partitions), recip'd
                    # there, then bounced through DRAM for a stride-0
                    # partition broadcast back to [64,512].  The chain runs
                    # on DVE + DGE queues only and is deferred, so the PE
                    # never waits on it; the last chunk's chain rides the
                    # then-idle scalar hwdge queue.
                    last = (j == NJ - 1 and m == 1)
                    # the last chain is latency-exposed: split the two heads
                    # across the scalar and sync hwdge queues so their hops
                    # overlap instead of serializing
                    dmaes = ((nc.scalar, nc.sync) if last
                             else (nc.gpsimd, nc.gpsimd))
                    dmae = dmaes[0]
                    caccs = [small.tile([VBLK, CH], f32, tag="cacc",
                                        name="cacc") for _ in range(2)]
                    for hh in range(2):
                        nc.vector.tensor_copy(caccs[hh][:], accs[hh][:VBLK, :])
                    nq = CH // P
                    rq = small.tile([P, 2 * nq], f32, tag="rq", name="rq")
                    for hh in range(2):
                        dmaes[hh].dma_start(
                            rq[:, hh * nq:(hh + 1) * nq],
                            caccs[hh][HD:HD + 1, :].rearrange(
                                "o (p c) -> o p c", p=P))
                    rr = small.tile([P, 2 * nq], f32, tag="rr", name="rr")
                    nc.vector.reciprocal(rr[:], rq[:])
                    drb = drbp.tile([2, CH], f32, tag="drb", name="drb")
                    for hh in range(2):
                        dmaes[hh].dma_start(
                            drb[hh:hh + 1, :].rearrange(
                                "o (p c) -> (o p) c", p=P),
                            rr[:, hh * nq:(hh + 1) * nq])
                    bbs = []
                    for hh in range(2):
                        bb = bpool.tile([HD, CH], f32, tag="bb", name="bb")
                        dmaes[hh].dma_start(
                            bb[:], drb[hh:hh + 1, :].broadcast_to([HD, CH]))
                        bbs.append(bb)
                    nc.vector.tensor_mul(ctxT[m][0:HD, jc],
                                         caccs[0][0:HD, :], bbs[0][:])
                    tmp = small.tile([HD, CH], MM_DT, tag="tmp", name="tmp")
                    nc.vector.tensor_mul(tmp[:], caccs[1][0:HD, :], bbs[1][:])
                    dmae.dma_start(ctxT[m][HD:P, jc], tmp[:])
                    if m == 1:
                        # ctxT for chunk j complete: its out-proj may now be
                        # scheduled (pops during the next chunk's loop).
                        # Half of chunk NJ-2's units are deferred to the
                        # drain, where they fill the last norm chain's DMA
                        # latency (the PE queue otherwise blocks ~13us on it).
                        for si in range(CH // P):
                            for nh in range(2):
                                u = emit_outproj_unit(
                                    j, si, nh, tail=(j == NJ - 1),
                                    fill=(j == NJ - 2 and 2 * si + nh >= 4))
                                if j == NJ - 2 and 2 * si + nh >= 4:
                                    tail_fill.append(u)
                                else:
                                    sprinkle.append(u)
                    return True
                pvq.append(emit_norm)

        # ---- drain: remaining PV/normalize, then the deferred units ----
        while pvq:
            pvq.popleft()()
        while tail_fill:
            tail_fill.popleft()()
        while sprinkle:
            sprinkle.popleft()()


_NC_CACHE = {}


def _get_program():
    if "nc" not in _NC_CACHE:
        _NC_CACHE["nc"] = _build_program()
    return _NC_CACHE["nc"]


def _make_in_maps(inputs):
    query = np.asarray(inputs["query"], dtype=np.float32)
    key = np.asarray(inputs["key"], dtype=np.float32)
    value = np.asarray(inputs["value"], dtype=np.float32)
    Wq = np.asarray(inputs["Wq"], dtype=np.float32)
    Wk = np.asarray(inputs["Wk"], dtype=np.float32)
    Wv = np.asarray(inputs["Wv"], dtype=np.float32)
    Wo = np.asarray(inputs["Wo"], dtype=np.float32)
    bq = np.asarray(inputs["bq"], dtype=np.float32)
    bk = np.asarray(inputs["bk"], dtype=np.float32)

    def xlayout(x):
        # [S, D] -> [P, NJ*KT*CH]: per partition, chunk-major with 8KB
        # contiguous runs (see dram_tensor comment in _build_program)
        a = x.T.reshape(KT, P, NJ, CH).transpose(1, 2, 0, 3)
        return np.ascontiguousarray(a).reshape(P, NJ * KT * CH).astype(NP_MM)

    def wlayout(w):
        # [D, M] -> [P, KT*M]: per-partition contiguous
        m = w.shape[1]
        a = w.reshape(KT, P, m).transpose(1, 0, 2)
        return np.ascontiguousarray(a).reshape(P, KT * m).astype(NP_MM)

    xT = {}
    for b in range(B):
        xT[("q", b)] = xlayout(query[b])
        xT[("k", b)] = xlayout(key[b])
        xT[("v", b)] = xlayout(value[b])

    in_maps = []
    for c in range(NCORE):
        b, g = divmod(c, G)
        cols = slice(g * DG, (g + 1) * DG)
        in_maps.append({
            "xqT": xT[("q", b)],
            "xkT": xT[("k", b)],
            "xvT": xT[("v", b)],
            "wq": wlayout(Wq[:, cols]),
            "wk": wlayout(Wk[:, cols]),
            "wv": wlayout(Wv[:, cols]),
            "wo": Wo[cols, :].reshape(2, P, D).transpose(1, 0, 2).reshape(
                P, 2 * D).astype(NP_MM),
            "bqk": np.ascontiguousarray(np.stack([bq[cols], bk[cols]])),
        })
    return in_maps


def kernel(query, key, value, Wq, bq, Wk, bk, Wv, bv, Wo, bo):
    bv = np.asarray(bv, dtype=np.float32)
    bo = np.asarray(bo, dtype=np.float32)
    Wo = np.asarray(Wo, dtype=np.float32)

    nc = _get_program()
    in_maps = _make_in_maps({
        "query": query, "key": key, "value": value, "Wq": Wq, "Wk": Wk,
        "Wv": Wv, "Wo": Wo, "bq": bq, "bk": bk,
    })

    res = run_bass_kernel_spmd(nc, in_maps, list(range(NCORE)))

    # unshard: sum the 4 head-group partials per batch; add the linear bias
    # correction (bv and bo commute through attention/out_proj).
    corr = bv @ Wo + bo
    out = np.empty((B, S, D), dtype=np.float32)
    for b in range(B):
        acc = res.results[4 * b]["out"].copy()
        for g in range(1, G):
            acc += res.results[4 * b + g]["out"]
        out[b] = acc + corr
    return out
